# revision 15
# baseline (speedup 1.0000x reference)
"""Cross-attention kernel for Trainium2 (8 NeuronCores, SPMD data-parallel).

Problem: O = softmax(Q @ K^T) @ V with B=4, Lq=Lk=4096, D=64, fp32 (no
1/sqrt(d) scaling).

Sharding: 8 cores = 4 batches x 2 Lq-halves. Each core handles a
[2048, 64] Q shard against the full [4096, 64] K/V of its batch.
Independent outputs -> no collectives.

Per-core algorithm (v3 — dual-engine exp, flat pipeline):
  - Host supplies QT [64, 2048] fp16 PRE-SCALED by 1/256 (exact: power of
    two) and KT [64, 4096] fp16; loaded once and duplicated across the
    upper partition half by on-chip SBUF->SBUF DMA so two k-chunks' score
    matmuls run row-tiled concurrently in the PE.
  - VA [128k, 32c, 128] bf16, host-prearranged contiguous: column 0 is
    the ones-column (softmax denominator), columns 64:128 are V; 1:64 are
    zero pad so the numerator rows start at partition 64 (engine APs
    wider than 32 partitions must start at 0/64, and custom DVE ops
    misread nonzero partition offsets, pinning the denominator to row 0).
  - ST[k, q] = matmul -> PSUM holds s/256. exp is split across BOTH the
    Scalar and Vector engines (the baseline was scalar-bound):
      * Scalar: activation(Exp, scale=256) -> bf16 PT.
      * Vector: two 8-stage custom DVE ops registered at runtime:
        p = poly4(t) ~ e^t (minimax c0=c1=1, rel err 1.05e-6, |t|<=.25),
        then p^256 via 8 chained squarings -> bf16 PT. (Src1 past stage 5
        crashes the DVE; the c1=1 constraint keeps Src1 out of the body.)
    Chunks are assigned greedily by modeled per-tile cost (scalar
    ~1.05us, DVE ~2.46us per [128,1024] tile).
  - OT[128, q] += matmul(lhsT=VA chunk [128, 128], rhs=PT [128, 512]);
    row 0 accumulates the denominator, rows 64:128 the numerator.
    The whole schedule is one flat pipeline across both q-blocks: score
    matmuls run a pair ahead, PV matmuls are emitted in predicted
    exp-completion order, and the q-block boundary interleaves (qb0's
    norm is emitted two pairs into qb1 — after qb1's first scores, but
    before qb1's first PV so the WAR edge on the single OT buffer holds).
  - Normalize: DVE fast-reciprocal of OT row 0, gpsimd broadcast, DVE
    multiply (rows 64:128), in 2 pieces of 512 columns; DMA out.
"""

import sys

for _p in ("/opt/trn_rl_repo", "/opt/pypackages"):
    if _p not in sys.path:
        sys.path.insert(0, _p)

from contextlib import ExitStack

import ml_dtypes
import numpy as np

import concourse.bacc as bacc
import concourse.mybir as mybir
import concourse.tile as tile
from concourse.bass_utils import run_bass_kernel_spmd

import concourse.dve_ops as _dve_ops
from concourse.dve_spec import C0, C1, C2, One, Spec, Src0, lower
from concourse.dve_spec import _has_src1
from concourse.dve_uop import DveOpSpec

# ---------------------------------------------------------------- constants
B, LQ, LK, D = 4, 4096, 4096, 64
N_CORES = 8
LQ_SHARD = LQ * B // N_CORES  # 2048
QB = 1024  # q-block (exp instruction free-size; 2 PSUM banks)
NQB = LQ_SHARD // QB  # 2
KC = 128  # k-chunk (contraction tile for the PV matmul)
NKC = LK // KC  # 32
NPAIR = NKC // 2  # 16 row-tiled score-matmul pairs per q-block
SL = 512  # matmul moving-dim slice (one PSUM bank)
NSL = QB // SL  # 2
SCORE_SCALE = 256.0  # host pre-scales Q by 1/256 (exact)
VA_W = 128  # VA columns: [ones | 63 zero pad | V]
V_OFF = 64  # V starts at column 64 (>=33-wide accesses must start at 0/64)

F32 = mybir.dt.float32
F16 = mybir.dt.float16
BF16 = mybir.dt.bfloat16
BF16NP = ml_dtypes.bfloat16

# Modeled per-[128,1024]-tile exp costs (us) for the greedy split.
TS = 1.05  # scalar: 1024/1.2GHz + access + seq
TD = 2.46  # DVE: two 1x passes + access + seq
TD_HANDICAP = 2.7  # norm work (recip+mul) of the previous q-block on DVE

# Minimax (c2, c3, c4) for e^t, t in [-0.25, 0.25], c0=c1=1 fixed.
EXP_C2 = 0.5000139854903264
EXP_C3 = 0.16711872930830435
EXP_C4 = 0.04146165926052129

# ------------------------------------------------- custom DVE exp ops
_t = Src0
_POLY_BODY = (((_t * C2 + C1) * _t + C0) * _t + One) * _t + One


def _poly_ref(in0, in1, s0, s1, imm2):
    x = in0.astype(np.float32)
    return ((((x * imm2 + s1) * x + s0) * x + 1.0) * x + 1.0).astype(np.float32)


_x = Src0
for _ in range(8):
    _x = _x * _x


def _sq_ref(in0, in1, s0, s1, imm2):
    return (in0.astype(np.float64) ** 256).astype(np.float32)


def _register_exp_ops():
    existing = {op.name: op for op in _dve_ops.OPS}
    if "EXP_POLY_V2_ANT" in existing:
        return existing["EXP_POLY_V2_ANT"], existing["EXP_SQUARE8_ANT"]
    specs = {
        "EXP_POLY_V2_ANT": Spec(body=_POLY_BODY, reference=_poly_ref),
        "EXP_SQUARE8_ANT": Spec(body=_x, reference=_sq_ref),
    }
    ops = []
    for name, spec in specs.items():
        row = max(_dve_ops._SUB_OPCODE_FOR_NAME.values()) + 1
        assert row < 0x20, "opcode row field overflow"
        _dve_ops._SUB_OPCODE_FOR_NAME[name] = row
        shas = {}
        for ver in ("v3", "v4"):
            try:
                tmp = DveOpSpec(
                    name=name,
                    opcode=row,
                    uops=lower(spec, ver=ver),
                    rd1_en=_has_src1(spec),
                )
                shas[ver] = tmp.sha(ver)
            except Exception:
                pass
        op = _dve_ops.DveOp(name, spec, subdim=False, uops_sha=shas)
        _dve_ops.OPS.append(op)
        _dve_ops.CUSTOM_DVE_SPECS[name] = spec
        ops.append(op)
    return ops[0], ops[1]


def _assign_engines():
    """Greedy chunk->engine split by modeled cost; returns list of 'S'/'D'."""
    import os

    ov = os.environ.get("KERNEL_ASSIGN", "")
    if ov == "ALL_S":
        return ["S"] * NKC
    t_s, t_d = 0.0, TD_HANDICAP
    out = []
    for _c in range(NKC):
        if t_s + TS <= t_d + TD:
            out.append("S")
            t_s += TS
        else:
            out.append("D")
            t_d += TD
    # Avoid two DVE tiles alive per pair (slow serial polys).
    for p in range(NPAIR):
        if out[2 * p] == "D" and out[2 * p + 1] == "D":
            out[2 * p + 1] = "S"
    return out


def _pv_order(assign):
    """Chunk indices sorted by predicted exp completion time."""
    done = {}
    t_s, t_d = 0.0, TD_HANDICAP
    for c, eng in enumerate(assign):
        if eng == "S":
            t_s += TS
            done[c] = t_s
        else:
            t_d += TD
            done[c] = t_d
    return sorted(range(NKC), key=lambda c: (done[c], c))


# ---------------------------------------------------------------- program
def _build_program():
    poly_op, sq_op = _register_exp_ops()

    nc = bacc.Bacc(
        "TRN2",
        target_bir_lowering=False,
        debug=False,
        num_devices=N_CORES,
    )
    qt_d = nc.declare_dram_parameter("QT", [D, LQ_SHARD], F16, isOutput=False)
    kt_d = nc.declare_dram_parameter("KT", [D, LK], F16, isOutput=False)
    va_d = nc.declare_dram_parameter(
        "VA", [KC, NKC * VA_W], BF16, isOutput=False
    )
    ot_d = nc.declare_dram_parameter("OT", [D, LQ_SHARD], F32, isOutput=True)

    assign = _assign_engines()
    pv_order = _pv_order(assign)

    with tile.TileContext(nc) as tc, ExitStack() as ctx:
        singles = ctx.enter_context(tc.tile_pool(name="singles", bufs=1))
        st_pool = ctx.enter_context(
            tc.tile_pool(name="st", bufs=3, space="PSUM")
        )
        ot_pool = ctx.enter_context(
            tc.tile_pool(name="ot", bufs=1, space="PSUM")
        )
        pt_pool = ctx.enter_context(tc.tile_pool(name="pt", bufs=6))
        scr_pool = ctx.enter_context(tc.tile_pool(name="scr", bufs=2))
        out_pool = ctx.enter_context(tc.tile_pool(name="out", bufs=4))
        norm_pool = ctx.enter_context(tc.tile_pool(name="norm", bufs=4))

        # Preload the exp activation table while input DMAs run.
        warm = singles.tile([1, 2], F32)
        nc.vector.memset(warm[:, :], 0.0)
        nc.scalar.activation(
            out=warm[:, :], in_=warm[:, :],
            func=mybir.ActivationFunctionType.Exp,
        )

        # Inputs: load rows 0:64 from DRAM, duplicate to rows 64:128 with
        # on-chip SBUF->SBUF DMA (halves DRAM traffic). kt in 4 pieces so
        # the first score matmul starts early; first-needed pieces first.
        KP = 1024  # kt DMA piece width (cols); deps are tile-granular, so
        NKP = LK // KP  # each piece is its own tile to unblock early scores
        VH = NKC // 2
        qt_sb = [
            singles.tile([2 * D, QB], F16, name=f"qt{h}") for h in range(NQB)
        ]
        kt_sb = [
            singles.tile([2 * D, KP], F16, name=f"kt{p}") for p in range(NKP)
        ]
        va_sb = [
            singles.tile([KC, VH, VA_W], BF16, name=f"va{h}") for h in range(2)
        ]

        def load_kt_piece(piece):
            sl_d = slice(piece * KP, (piece + 1) * KP)
            nc.sync.dma_start(out=kt_sb[piece][0:D, :], in_=kt_d[:, sl_d])
            nc.sync.dma_start(
                out=kt_sb[piece][D : 2 * D, :], in_=kt_sb[piece][0:D, :]
            )

        def load_qt(qb):
            sl_d = slice(qb * QB, (qb + 1) * QB)
            nc.sync.dma_start(out=qt_sb[qb][0:D, :], in_=qt_d[:, sl_d])
            nc.sync.dma_start(
                out=qt_sb[qb][D : 2 * D, :], in_=qt_sb[qb][0:D, :]
            )

        load_kt_piece(0)
        load_qt(0)
        for p_ in range(1, NKP):
            load_kt_piece(p_)
        load_qt(1)
        for h in range(2):
            nc.sync.dma_start(
                out=va_sb[h][:, :, :],
                in_=va_d[:, h * VH * VA_W : (h + 1) * VH * VA_W].rearrange(
                    "p (c w) -> p c w", w=VA_W
                ),
            )

        def kt_ap(half, c):
            t = kt_sb[c * KC // KP]
            off = (c * KC) % KP
            return t[half * D : (half + 1) * D, off : off + KC]

        def va_ap(c):
            return va_sb[c // VH][:, c % VH, :]

        # -------- flat software pipeline across both q-blocks
        qb_state = [
            {"ot": None, "st": {}, "pt": {}, "pv_pos": 0} for _ in range(NQB)
        ]
        norm_emitted = [False] * NQB

        def emit_scores(qb, p):
            st = qb_state[qb]["st"]
            c0, c1 = 2 * p, 2 * p + 1
            st_a = st_pool.tile([KC, QB], F32, tag="st")
            st_b = st_pool.tile([KC, QB], F32, tag="st")
            st[c0], st[c1] = st_a, st_b
            qt = qt_sb[qb]
            for s in range(NSL):
                sl = slice(s * SL, (s + 1) * SL)
                nc.tensor.matmul(
                    out=st_a[:, sl],
                    lhsT=kt_ap(0, c0),
                    rhs=qt[0:D, sl],
                    start=True,
                    stop=True,
                    tile_position=(0, 0),
                )
                nc.tensor.matmul(
                    out=st_b[:, sl],
                    lhsT=kt_ap(1, c1),
                    rhs=qt[D : 2 * D, sl],
                    start=True,
                    stop=True,
                    tile_position=(D, 0),
                )

        def emit_exps(qb, p):
            s = qb_state[qb]
            for c in (2 * p, 2 * p + 1):
                pt = pt_pool.tile([KC, QB], BF16)
                s["pt"][c] = pt
                if assign[c] == "S":
                    nc.scalar.activation(
                        out=pt[:, :],
                        in_=s["st"][c][:, :],
                        func=mybir.ActivationFunctionType.Exp,
                        scale=SCORE_SCALE,
                    )
                else:
                    scr = scr_pool.tile([KC, QB], F32)
                    nc.vector._custom_dve(
                        poly_op,
                        out=scr[:, :],
                        in0=s["st"][c][:, :],
                        s0=EXP_C2,
                        s1=EXP_C3,
                        imm2=EXP_C4,
                    )
                    nc.vector._custom_dve(sq_op, out=pt[:, :], in0=scr[:, :])

        def emit_pvs(count):
            """Emit up to `count` PV chunk-jobs from the oldest q-block with
            pending eligible PVs (exp emitted; qb>0 needs the previous
            q-block's norm emitted so the OT WAR edge exists)."""
            n = 0
            for qb in range(NQB):
                s = qb_state[qb]
                if qb > 0 and not norm_emitted[qb - 1]:
                    break
                while n < count and s["pv_pos"] < NKC:
                    c = pv_order[s["pv_pos"]]
                    if c not in s["pt"]:
                        break
                    if s["ot"] is None:
                        s["ot"] = ot_pool.tile([VA_W, QB], F32, name="ot_ps")
                    pt = s["pt"][c]
                    for sli in range(NSL):
                        sl = slice(sli * SL, (sli + 1) * SL)
                        nc.tensor.matmul(
                            out=s["ot"][:, sl],
                            lhsT=va_ap(c),
                            rhs=pt[:, sl],
                            start=(s["pv_pos"] == 0),
                            stop=(s["pv_pos"] == NKC - 1),
                        )
                    s["pv_pos"] += 1
                    n += 1
                if s["pv_pos"] < NKC:
                    break  # strict q-block order for PV emission

        def emit_norm(qb):
            ot_ps = qb_state[qb]["ot"]
            for piece in range(NSL):
                sl = slice(piece * SL, (piece + 1) * SL)
                recip = norm_pool.tile([1, SL], F32)
                nc.vector.reciprocal_approx_fast(recip[:, :], ot_ps[0:1, sl])
                bcast = norm_pool.tile([D, SL], F32)
                nc.gpsimd.partition_broadcast(bcast[:, :], recip[:, :])
                o_sb = out_pool.tile([D, SL], F32)
                nc.vector.tensor_mul(
                    o_sb[:, :], ot_ps[V_OFF : V_OFF + D, sl], bcast[:, :]
                )
                nc.sync.dma_start(
                    out=ot_d[
                        :, qb * QB + piece * SL : qb * QB + (piece + 1) * SL
                    ],
                    in_=o_sb[:, :],
                )
            norm_emitted[qb] = True

        for g in range(NQB * NPAIR):
            qb, p = divmod(g, NPAIR)
            if g >= 2:
                emit_pvs(2)
            emit_scores(qb, p)
            emit_exps(qb, p)
            # Two pairs into each later q-block: drain the previous
            # q-block's PVs and emit its norm (before this block's PVs).
            if p == 1 and qb > 0:
                while qb_state[qb - 1]["pv_pos"] < NKC:
                    emit_pvs(NKC)
                emit_norm(qb - 1)
        emit_pvs(2 * NKC)  # drain everything
        emit_norm(NQB - 1)

    nc.finalize()
    return nc


_PROGRAM_CACHE = {}


def _get_program():
    if "nc" not in _PROGRAM_CACHE:
        _PROGRAM_CACHE["nc"] = _build_program()
    return _PROGRAM_CACHE["nc"]


def _make_in_maps(Q, K, V):
    Q = np.asarray(Q, dtype=np.float32)
    K = np.asarray(K, dtype=np.float32)
    V = np.asarray(V, dtype=np.float32)
    in_maps = []
    for core in range(N_CORES):
        b, half = core // 2, core % 2
        q_shard = Q[b, half * LQ_SHARD : (half + 1) * LQ_SHARD, :]
        qt = np.ascontiguousarray(q_shard.T / SCORE_SCALE).astype(np.float16)
        kt = np.ascontiguousarray(K[b].T).astype(np.float16)
        # VA: [ones | 63 zero pad | V], prearranged to [KC, NKC*VA_W] so
        # the DMA is contiguous 2KB+ rows per partition.
        va = np.zeros((LK, VA_W), dtype=np.float32)
        va[:, 0] = 1.0
        va[:, V_OFF:] = V[b]
        va = (
            va.reshape(NKC, KC, VA_W)
            .transpose(1, 0, 2)
            .reshape(KC, NKC * VA_W)
        )
        in_maps.append(
            {
                "QT": qt,
                "KT": kt,
                "VA": np.ascontiguousarray(va).astype(BF16NP),
            }
        )
    return in_maps


def _run(Q, K, V, trace=False, **spmd_kwargs):
    nc = _get_program()
    in_maps = _make_in_maps(Q, K, V)
    res = run_bass_kernel_spmd(
        nc, in_maps, list(range(N_CORES)), trace=trace, **spmd_kwargs
    )
    out = np.empty((B, LQ, D), dtype=np.float32)
    for core in range(N_CORES):
        b, half = core // 2, core % 2
        ot = res.results[core]["OT"]  # [64, 2048]
        out[b, half * LQ_SHARD : (half + 1) * LQ_SHARD, :] = ot.T
    return out, res


def kernel(Q, K, V):
    out, _ = _run(Q, K, V, trace=False)
    return out
